# revision 1
# baseline (speedup 1.0000x reference)
"""Causal single-head attention (B=4, T=4096, C=512, D=64) on 8 TRN2 NeuronCores.

Sharding: core c -> (batch b = c // 2, parity P = c % 2).  Each batch's 32
q-tiles (128 rows each) are striped by parity: core (b, P) owns global q-tiles
k = 2j + P, j = 0..15.  Slot j's causal kv extent is padded to 256*(j+1) keys
(uniform across parities, +3% work) and the last 256 key columns get a
parity-specific additive mask fed as input data, so one SPMD program serves
all 8 cores.

Per-core dataflow:
  phase 1: K^T|V^T from a single W-stationary matmul per x^T chunk (Wk and Wv
           stacked into one 128-wide stationary operand; C=512 contracted in
           4 chunks of 128); V^T is PE-transposed into fp32 V[S, D+1] tiles
           whose last column is 1.0 (gives row sums for free during AV).
           Q^T/K^T are mirrored into the upper 64 SBUF partitions so the
           scores matmuls run 2x row-tiled (K=64 uses half the PE array; two
           concurrent 64-row tiles).  K-hat = [K^T; ones] and Q-hat =
           [Q^T; -m_row] buffers serve the transposed-scores matmul.
  phase 2 (flash, per slot j, groups of up to 1536 keys):
           1. scores S = Q_j K^T on PE (row-tiled, 512-wide PSUM chunks),
              additive -1e30 mask on the slot's last 256 columns, DVE row-max
              -> running max m (these scores are used ONLY for the max).
           2. -m written as a [1,128] row into Q-hat partition 64 (PE
              transpose of m via identity matmul + DVE negate-copy).
           3. S^T - m computed directly on PE via the 65-deep contraction
              [K^T; 1]^T [Q^T; -m] -> one ACT exp (scale=8) writes A^T
              straight to SBUF (no PSUM->SBUF copy pass, no PE transposes
              of A), masked via a transposed mask on the last two blocks.
           4. AV: po[128, 65] += A^T_block^T V-hat_block on PE; column 64
              accumulates the row sums l.  Running rescale of (O, l) by
              exp(8*(m_old - m_new)) on DVE; final y = O / l.
"""

import numpy as np

B, T, C, D = 4, 4096, 512, 64
P128 = 128
NSLOT = 16          # q-tile slots per core
TQ = NSLOT * P128   # 2048 q rows per core
NEG = -1.0e30
GRP = 1536

_CACHED = {}


def _build(use_dma_t=False, rowtile=True, st_mode=True, grp=GRP):
    import concourse.bass as bass
    import concourse.mybir as mybir
    from concourse import bacc
    from concourse.tile import TileContext
    from concourse.masks import make_identity

    f32 = mybir.dt.float32
    bf16 = mybir.dt.bfloat16
    AX = mybir.AxisListType.X
    ALU = mybir.AluOpType
    ACTF = mybir.ActivationFunctionType

    nc = bacc.Bacc("TRN2", target_bir_lowering=False, debug=False,
                   enable_asserts=False, num_devices=8)

    xT = nc.dram_tensor("xT", [C, T], f32, kind="ExternalInput").ap()
    xTq = nc.dram_tensor("xTq", [C, TQ], f32, kind="ExternalInput").ap()
    wq = nc.dram_tensor("wq", [C, D], f32, kind="ExternalInput").ap()
    wk = nc.dram_tensor("wk", [C, D], f32, kind="ExternalInput").ap()
    wv = nc.dram_tensor("wv", [C, D], f32, kind="ExternalInput").ap()
    bq = nc.dram_tensor("bq", [D, 1], f32, kind="ExternalInput").ap()
    bk = nc.dram_tensor("bk", [D, 1], f32, kind="ExternalInput").ap()
    bv = nc.dram_tensor("bv", [D, 1], f32, kind="ExternalInput").ap()
    maskp = nc.dram_tensor("maskp", [P128, 512], f32, kind="ExternalInput").ap()
    maskpT = nc.dram_tensor("maskpT", [P128, 256], f32, kind="ExternalInput").ap()
    y = nc.dram_tensor("y", [TQ, D], f32, kind="ExternalOutput").ap()
    DV = D + 1 if st_mode else D  # V tiles carry a ones column in st_mode

    with TileContext(nc) as tc:
        with (
            tc.tile_pool(name="singles", bufs=1) as singles,
            tc.tile_pool(name="xin", bufs=3) as xin,
            tc.tile_pool(name="work", bufs=2) as work,
            tc.tile_pool(name="small", bufs=3) as small,
            tc.tile_pool(name="ps_s", bufs=2, space="PSUM") as ps_s,
            tc.tile_pool(name="ps_s2", bufs=1, space="PSUM") as ps_s2,
            tc.tile_pool(name="ps_t", bufs=1, space="PSUM") as ps_t,
            tc.tile_pool(name="ps_o", bufs=1, space="PSUM") as ps_o,
            tc.tile_pool(name="ps_p", bufs=1, space="PSUM") as ps_p,
        ):
            # ---- resident constants (SWDGE loads) ----
            wqs = singles.tile([P128, 4, D], f32, tag="wqs")
            wkv = singles.tile([P128, 4, 2 * D], f32, tag="wkv")
            nc.gpsimd.dma_start(out=wqs, in_=wq.rearrange("(c p) d -> p c d", p=P128))
            nc.gpsimd.dma_start(out=wkv[:, :, :D],
                                in_=wk.rearrange("(c p) d -> p c d", p=P128))
            nc.gpsimd.dma_start(out=wkv[:, :, D:],
                                in_=wv.rearrange("(c p) d -> p c d", p=P128))
            bqs = singles.tile([D, 1], f32, tag="bqs")
            bks = singles.tile([D, 1], f32, tag="bks")
            bvs = singles.tile([D, 1], f32, tag="bvs")
            nc.gpsimd.dma_start(out=bqs, in_=bq)
            nc.gpsimd.dma_start(out=bks, in_=bk)
            nc.gpsimd.dma_start(out=bvs, in_=bv)
            msk = singles.tile([P128, 512], f32, tag="msk")
            nc.gpsimd.dma_start(out=msk, in_=maskp)
            if st_mode:
                mskT = singles.tile([P128, 256], f32, tag="mskT")
                nc.gpsimd.dma_start(out=mskT, in_=maskpT)
                identf = singles.tile([P128, P128], f32, tag="identf")
                make_identity(nc, identf)
            else:
                mskT = identf = None

            QP = P128 if rowtile else D
            QTo = singles.tile([QP, TQ], f32, tag="QTo")
            KT = singles.tile([QP, T], f32, tag="KT")
            Vsb = singles.tile([P128, (T // P128) * DV], f32, tag="Vsb")
            if st_mode:
                # K-hat: [K^T; ones] and Q-hat: [Q^T; -m_row] for the shifted
                # transposed-scores matmul (rank-1 max subtraction in-contraction)
                KH = singles.tile([D + 1, T], f32, tag="KH")
                QH = singles.tile([D + 1, TQ], f32, tag="QH")
                nc.vector.memset(KH[D:D + 1, :], 1.0)
                nc.vector.memset(Vsb, 1.0)   # ones column survives V writes

            # ---- phase 1: projections ----
            for t8 in range(T // 512):
                xt = xin.tile([P128, 4, 512], f32, tag="xt")
                nc.gpsimd.dma_start(
                    out=xt,
                    in_=xT[:, t8 * 512:(t8 + 1) * 512].rearrange(
                        "(c p) n -> p c n", p=P128),
                )
                kvps = ps_p.tile([2 * D, 512], f32, tag="pp")
                for c in range(4):
                    nc.tensor.matmul(kvps, wkv[:, c, :], xt[:, c, :],
                                     start=(c == 0), stop=(c == 3))
                nc.scalar.activation(KT[:D, t8 * 512:(t8 + 1) * 512], kvps[:D, :],
                                     ACTF.Identity, bias=bks, scale=1.0)
                vtmp = work.tile([D, 512], f32, tag="vtmp")
                nc.scalar.activation(vtmp, kvps[D:, :], ACTF.Identity,
                                     bias=bvs, scale=1.0)
                # transpose V^T [64, 128] blocks -> bf16 V [128, 64] tiles
                for i in range(4):
                    t = t8 * 4 + i
                    if use_dma_t:
                        nc.sync.dma_start(
                            out=Vsb[:, t * DV:t * DV + D],
                            in_=vtmp[:, i * P128:(i + 1) * P128], transpose=True)
                    else:
                        pt = ps_t.tile([P128, 512], f32, tag="pt")
                        nc.tensor.transpose(pt[:, :D],
                                            vtmp[:, i * P128:(i + 1) * P128],
                                            identf[:D, :D])
                        nc.vector.tensor_copy(Vsb[:, t * DV:t * DV + D], pt[:, :D])
            # Q^T from xTq (own 2048 rows)
            for t8 in range(TQ // 512):
                xt = xin.tile([P128, 4, 512], f32, tag="xt")
                nc.gpsimd.dma_start(
                    out=xt,
                    in_=xTq[:, t8 * 512:(t8 + 1) * 512].rearrange(
                        "(c p) n -> p c n", p=P128),
                )
                qps = ps_p.tile([D, 512], f32, tag="pp")
                for c in range(4):
                    nc.tensor.matmul(qps, wqs[:, c, :], xt[:, c, :],
                                     start=(c == 0), stop=(c == 3))
                nc.scalar.activation(QTo[:D, t8 * 512:(t8 + 1) * 512], qps,
                                     ACTF.Identity, bias=bqs, scale=1.0)
            if rowtile:
                # mirror Q^T/K^T into the upper 64 partitions for row tiling
                nc.gpsimd.dma_start(out=KT[D:2 * D, :], in_=KT[:D, :])
                nc.gpsimd.dma_start(out=QTo[D:2 * D, :], in_=QTo[:D, :])
            if st_mode:
                nc.gpsimd.dma_start(out=KH[:D, :], in_=KT[:D, :])
                nc.gpsimd.dma_start(out=QH[:D, :], in_=QTo[:D, :])

            # ---- phase 2: per-slot flash attention ----
            for j in range(NSLOT):
                ncols = 256 * (j + 1)
                groups = []
                off = 0
                while off < ncols:
                    groups.append((off, min(grp, ncols - off)))
                    off += grp

                mrun = small.tile([P128, 1], f32, tag="mrun")
                lrun = small.tile([P128, 1], f32, tag="lrun")
                Oacc = small.tile([P128, D], f32, tag="Oacc")

                for gi, (off, w) in enumerate(groups):
                    last = (gi == len(groups) - 1)
                    subs = list(range(0, w, 512))
                    mgp = small.tile([P128, 4], f32, tag="mgp")
                    for si, soff in enumerate(subs):
                        sw = min(512, w - soff)
                        half = ((off + soff) // 512) % 2 if rowtile else 0
                        pbase = half * D
                        ps = ps_s.tile([P128, 512], f32, tag="ps")
                        nc.tensor.matmul(
                            ps[:, :sw],
                            QTo[pbase:pbase + D, j * P128:(j + 1) * P128],
                            KT[pbase:pbase + D, off + soff:off + soff + sw],
                            start=True, stop=True)
                        if last and si == len(subs) - 1:
                            nc.vector.tensor_add(ps[:, sw - 256:sw],
                                                 ps[:, sw - 256:sw],
                                                 msk[:, 256:512])
                        nc.vector.reduce_max(mgp[:, si:si + 1], ps[:, :sw], axis=AX)
                    mg = small.tile([P128, 1], f32, tag="mg")
                    if len(subs) > 1:
                        nc.vector.reduce_max(mg, mgp[:, :len(subs)], axis=AX)
                    else:
                        nc.vector.tensor_copy(mg, mgp[:, :1])
                    if gi == 0:
                        nc.vector.tensor_copy(mrun, mg)
                    else:
                        mnew = small.tile([P128, 1], f32, tag="mnew")
                        nc.vector.tensor_max(mnew, mrun, mg)
                        mdiff = small.tile([P128, 1], f32, tag="mdiff")
                        nc.vector.tensor_sub(mdiff, mrun, mnew)
                        cstep = small.tile([P128, 1], f32, tag="cstep")
                        nc.scalar.activation(cstep, mdiff, ACTF.Exp,
                                             bias=0.0, scale=8.0)
                        nc.vector.tensor_copy(mrun, mnew)
                    nblk = w // P128
                    base = off // P128
                    AT = work.tile([P128, grp], f32, tag="AT")
                    po = ps_o.tile([P128, DV], f32, tag="po")
                    if st_mode:
                        # -m_new as a [1, 128] row at QH partition 64 (via PE)
                        pm = ps_t.tile([P128, P128], f32, tag="pt")
                        nc.tensor.matmul(pm[:1, :P128], mrun, identf,
                                         start=True, stop=True)
                        nc.vector.tensor_scalar_mul(
                            QH[D:D + 1, j * P128:(j + 1) * P128],
                            pm[:1, :P128], -1.0)
                        # shifted transposed scores: S^T - m  (65-contraction)
                        ps2 = ps_s2.tile([P128, grp], f32, tag="ps2")
                        for i in range(nblk):
                            nc.tensor.matmul(
                                ps2[:, i * P128:(i + 1) * P128],
                                KH[:, off + i * P128:off + (i + 1) * P128],
                                QH[:, j * P128:(j + 1) * P128],
                                start=True, stop=True)
                        if last:
                            nc.vector.tensor_add(
                                ps2[:, w - 256:w - P128], ps2[:, w - 256:w - P128],
                                mskT[:, 0:P128])
                            nc.vector.tensor_add(
                                ps2[:, w - P128:w], ps2[:, w - P128:w],
                                mskT[:, P128:256])
                        nc.scalar.activation(AT[:, :w], ps2[:, :w], ACTF.Exp,
                                             bias=0.0, scale=8.0)
                    else:
                        raise NotImplementedError("non-st_mode path removed")
                    for i in range(nblk):
                        nc.tensor.matmul(po, AT[:, i * P128:(i + 1) * P128],
                                         Vsb[:, (base + i) * DV:(base + i) * DV + DV],
                                         start=(i == 0), stop=(i == nblk - 1))

                    lg_ap = po[:, D:D + 1]
                    if gi == 0:
                        nc.vector.tensor_copy(Oacc, po[:, :D])
                        nc.vector.tensor_copy(lrun, lg_ap)
                    else:
                        nc.vector.scalar_tensor_tensor(
                            out=Oacc, in0=Oacc, scalar=cstep, in1=po[:, :D],
                            op0=ALU.mult, op1=ALU.add)
                        nc.vector.scalar_tensor_tensor(
                            out=lrun, in0=lrun, scalar=cstep, in1=lg_ap,
                            op0=ALU.mult, op1=ALU.add)

                rl = small.tile([P128, 1], f32, tag="rl")
                nc.vector.reciprocal(rl, lrun)
                yt = small.tile([P128, D], f32, tag="yt")
                nc.vector.tensor_scalar_mul(yt, Oacc, rl)
                nc.gpsimd.dma_start(out=y[j * P128:(j + 1) * P128, :], in_=yt)

    nc.compile()
    return nc


def _get_nc():
    if "nc" not in _CACHED:
        _CACHED["nc"] = _build()
    return _CACHED["nc"]


def _prep_in_maps(x, Wq, bq, Wk, bk, Wv, bv):
    x = np.asarray(x, dtype=np.float32)
    Wq = np.asarray(Wq, dtype=np.float32)
    Wk = np.asarray(Wk, dtype=np.float32)
    Wv = np.asarray(Wv, dtype=np.float32)
    bq_ = np.asarray(bq, dtype=np.float32).reshape(D, 1)
    bk_ = np.asarray(bk, dtype=np.float32).reshape(D, 1)
    bv_ = np.asarray(bv, dtype=np.float32).reshape(D, 1)

    tri = np.triu(np.ones((P128, P128), np.float32), k=1) * NEG
    masks = []
    for P in range(2):
        mp = np.zeros((P128, 512), np.float32)
        if P == 0:
            mp[:, 256:384] = tri
            mp[:, 384:512] = NEG
        else:
            mp[:, 384:512] = tri
        masks.append(mp)

    masksT = []
    for P in range(2):
        mt = np.zeros((P128, 256), np.float32)
        mt[:, 0:128] = masks[P][:, 256:384].T
        mt[:, 128:256] = masks[P][:, 384:512].T
        masksT.append(mt)

    in_maps = []
    for c in range(8):
        b, P = c // 2, c % 2
        xb = x[b]                                   # [T, C]
        rows = (np.arange(NSLOT) * 2 + P)[:, None] * P128 + np.arange(P128)[None, :]
        rows = rows.reshape(-1)
        in_maps.append({
            "xT": np.ascontiguousarray(xb.T),
            "xTq": np.ascontiguousarray(xb[rows].T),
            "wq": Wq, "wk": Wk, "wv": Wv,
            "bq": bq_, "bk": bk_, "bv": bv_,
            "maskp": masks[P], "maskpT": masksT[P],
        })
    return in_maps


def _unshard(res):
    out = np.empty((B, T, D), np.float32)
    for c in range(8):
        b, P = c // 2, c % 2
        yl = res.results[c]["y"]
        for j in range(NSLOT):
            k = 2 * j + P
            out[b, k * P128:(k + 1) * P128] = yl[j * P128:(j + 1) * P128]
    return out


def kernel(x, Wq, bq, Wk, bk, Wv, bv):
    from concourse.bass_utils import run_bass_kernel_spmd

    in_maps = _prep_in_maps(x, Wq, bq, Wk, bk, Wv, bv)
    res = run_bass_kernel_spmd(_get_nc(), in_maps, core_ids=list(range(8)))
    _CACHED["last_results"] = res
    return _unshard(res)


def run_profiled(np_inputs):
    from concourse.bass_utils import run_bass_kernel_spmd

    in_maps = _prep_in_maps(**np_inputs)
    res = run_bass_kernel_spmd(_get_nc(), in_maps, core_ids=list(range(8)),
                               trace=True)
    _CACHED["last_results"] = res
    return res


if __name__ == "__main__":
    rng = np.random.default_rng(0)
    x = rng.standard_normal((B, T, C), dtype=np.float32)
    s = 1.0 / np.sqrt(C)
    Wq = rng.standard_normal((C, D), dtype=np.float32) * s
    Wk = rng.standard_normal((C, D), dtype=np.float32) * s
    Wv = rng.standard_normal((C, D), dtype=np.float32) * s
    z = np.zeros(D, np.float32)
    print(kernel(x, Wq, z, Wk, z, Wv, z).shape)



# revision 14
# speedup vs baseline: 1.9570x; 1.9570x over previous
"""Causal single-head attention (B=4, T=4096, C=512, D=64) on 8 TRN2 NeuronCores.

Sharding: core c -> (batch b = c // 2, parity P = c % 2).  Core (b, P) owns
global q-tiles g = 2j + P, j = 0..15 (16 slots of 128 rows).  Slots are
processed in PAIRS i = (2i, 2i+1) so the transposed-scores matmuls have a
256-wide moving dim (f32r needs >= 256 for 1 cyc/row).  The key axis is
stripe-swapped on the host for P=1 cores (each 256-key block's two 128-key
halves exchanged) so each core's own q columns sit at fixed offsets of every
512-key chunk; masks are input data, so one SPMD program serves all 8 cores.

Numerics: all big matmuls run in fp32r (TF32-like: 11-bit mantissa RTN, fp32
range, 1 PE cycle/row at moving-dim >= 256 vs 4 for fp32).  x and W are
pre-rounded to fp32r on the host; the projection adds a residual term
(bf16(x - round(x)) @ bf16(W)) to recover near-fp32 q/k/v (rel err ~1.1e-2
vs 2e-2 budget; single-rounding alone is 2.02e-2).  exp outputs fp16 A^T;
AV runs in fp16 with a ones column in V giving row sums for free.

Per-core dataflow:
  phase 1 (per 512-col chunk t8): xh (f32r) + xl (bf16 residual) DMA'd in;
    K|V projected via 8 accumulated matmuls (4 f32r + 4 bf16 residual);
    ACT writes K^T into KH[0:64] (f32r) and V^T into vtmp (fp16); vtmp is
    DMA-transposed (XBAR) into fp16 V[128, 65] tiles with a ones column.
    Q^T likewise from the chunk's own 256 q-columns into QH[0:64].
  phase 2 (per pair i, extent E = 512(i+1)):
    max: S = Q K^T per 512-key chunk (f32r, PSUM ring), DVE row-max
      (mask merged via tensor_tensor_reduce on the last chunk) -> m (bf16);
      PE transposes -m into QH[64] via negated-identity matmul + DVE copy.
    windows (512 keys): S^T - m via the 65-deep contraction
      [K^T;1]^T [Q^T;-m] (4 matmuls of [128k, 256q]); causal masks added ON
      PE by one extra matmul per boundary tile: TriL''^T M2 where
      TriL''[r,s] = (r < s or r == 127) and M2 carries -1e30 rows (input
      data); ACT exp (scale=8) -> fp16 A^T; AV: po[128, 2, 65] += A^T V
      accumulated across the whole pair (no flash rescale -- m is final).
    y = po[:, :, :64] / po[:, :, 64] (DVE reciprocal + scalar mul), DMA out.
"""

import numpy as np

B, T, C, D = 4, 4096, 512, 64
P128 = 128
NSLOT = 16
NPAIR = 8
TQ = NSLOT * P128   # 2048 q rows per core
NEG = -1.0e30

_CACHED = {}


def _round12(v):
    u = np.ascontiguousarray(v, np.float32).view(np.uint32)
    half = np.uint32(1 << 11)
    u2 = ((u + half) >> 12) << 12
    return u2.view(np.float32)


def _build():
    import concourse.mybir as mybir
    from concourse import bacc
    from concourse.tile import TileContext

    f32 = mybir.dt.float32
    f32r = mybir.dt.float32r
    bf16 = mybir.dt.bfloat16
    fp16 = mybir.dt.float16
    AX = mybir.AxisListType.X
    ALU = mybir.AluOpType
    ACTF = mybir.ActivationFunctionType

    nc = bacc.Bacc("TRN2", target_bir_lowering=False, debug=False,
                   enable_asserts=False, num_devices=8)

    xh_d = nc.dram_tensor("xh", [C, T], f32r, kind="ExternalInput").ap()
    xl_d = nc.dram_tensor("xl", [C, T], bf16, kind="ExternalInput").ap()
    wkvh_d = nc.dram_tensor("wkvh", [C, 2 * D], f32r, kind="ExternalInput").ap()
    wkvb_d = nc.dram_tensor("wkvb", [C, 2 * D], bf16, kind="ExternalInput").ap()
    wqh_d = nc.dram_tensor("wqh", [C, D], f32r, kind="ExternalInput").ap()
    wqb_d = nc.dram_tensor("wqb", [C, D], bf16, kind="ExternalInput").ap()
    bq_d = nc.dram_tensor("bq", [D, 1], f32, kind="ExternalInput").ap()
    bk_d = nc.dram_tensor("bk", [D, 1], f32, kind="ExternalInput").ap()
    bv_d = nc.dram_tensor("bv", [D, 1], f32, kind="ExternalInput").ap()
    m4_d = nc.dram_tensor("m4", [P128, 512], f32r, kind="ExternalInput").ap()
    trilq_d = nc.dram_tensor("trilq", [P128, P128], f32r, kind="ExternalInput").ap()
    m2_d = nc.dram_tensor("m2", [P128, 4, 256], f32r, kind="ExternalInput").ap()
    tril_d = nc.dram_tensor("tril", [P128, P128], f32r, kind="ExternalInput").ap()
    idn_d = nc.dram_tensor("idn", [P128, P128], bf16, kind="ExternalInput").ap()
    onek_d = nc.dram_tensor("onek", [1, T], f32r, kind="ExternalInput").ap()
    onev_d = nc.dram_tensor("onev", [P128, T // P128], fp16, kind="ExternalInput").ap()
    y_d = nc.dram_tensor("y", [TQ, D], f32, kind="ExternalOutput").ap()

    with TileContext(nc) as tc:
        with (
            tc.tile_pool(name="singles", bufs=1) as singles,
            tc.tile_pool(name="xin", bufs=8) as xin,
            tc.tile_pool(name="work", bufs=2) as work,
            tc.tile_pool(name="small", bufs=2) as small,
            tc.tile_pool(name="ps_p", bufs=1, space="PSUM") as ps_p,
            tc.tile_pool(name="ps_r", bufs=2, space="PSUM") as ps_r,
            tc.tile_pool(name="ps_s", bufs=2, space="PSUM") as ps_s,
            tc.tile_pool(name="ps_o", bufs=1, space="PSUM") as ps_o,
        ):
            # ---- resident constants ----
            wkvh = singles.tile([P128, 4, 2 * D], f32r, tag="wkvh")
            wkvb = singles.tile([P128, 4, 2 * D], bf16, tag="wkvb")
            wqh = singles.tile([P128, 4, D], f32r, tag="wqh")
            wqb = singles.tile([P128, 4, D], bf16, tag="wqb")
            nc.gpsimd.dma_start(out=wkvh, in_=wkvh_d.rearrange("(c p) d -> p c d", p=P128))
            nc.gpsimd.dma_start(out=wkvb, in_=wkvb_d.rearrange("(c p) d -> p c d", p=P128))
            nc.gpsimd.dma_start(out=wqh, in_=wqh_d.rearrange("(c p) d -> p c d", p=P128))
            nc.gpsimd.dma_start(out=wqb, in_=wqb_d.rearrange("(c p) d -> p c d", p=P128))
            bqs = singles.tile([D, 1], f32, tag="bqs")
            bks = singles.tile([D, 1], f32, tag="bks")
            bvs = singles.tile([D, 1], f32, tag="bvs")
            nc.gpsimd.dma_start(out=bqs, in_=bq_d)
            nc.gpsimd.dma_start(out=bks, in_=bk_d)
            nc.gpsimd.dma_start(out=bvs, in_=bv_d)
            m4 = singles.tile([P128, 512], f32r, tag="m4")
            nc.gpsimd.dma_start(out=m4, in_=m4_d)
            trilq = singles.tile([P128, P128], f32r, tag="trilq")
            nc.gpsimd.dma_start(out=trilq, in_=trilq_d)
            m2 = singles.tile([P128, 4, 256], f32r, tag="m2")
            nc.gpsimd.dma_start(out=m2, in_=m2_d)
            tril = singles.tile([P128, P128], f32r, tag="tril")
            nc.gpsimd.dma_start(out=tril, in_=tril_d)
            idn = singles.tile([P128, P128], bf16, tag="idn")
            nc.gpsimd.dma_start(out=idn, in_=idn_d)

            KH = singles.tile([D + 1, T], f32r, tag="KH")
            QH = singles.tile([D + 1, TQ], f32r, tag="QH")
            DV = 80  # V tile stride: 160B partition lines keep XBAR writes 32B-aligned
            Vsb = singles.tile([P128, (T // P128) * DV], fp16, tag="Vsb")
            Vv = Vsb.rearrange("p (t dv) -> p t dv", dv=DV)
            nc.gpsimd.dma_start(out=KH[D:D + 1, :], in_=onek_d)
            nc.gpsimd.dma_start(out=Vv[:, :, D], in_=onev_d)

            # ---- phase 1: one 512-col chunk of projections ----
            xtiles = []

            def emit_xload(t8):
                sl = slice(t8 * 512, (t8 + 1) * 512)
                xh = xin.tile([P128, 4, 512], f32r, tag="xh")
                xl = xin.tile([P128, 4, 512], bf16, tag="xl")
                nc.sync.dma_start(out=xh, in_=xh_d[:, sl].rearrange("(c p) n -> p c n", p=P128))
                nc.sync.dma_start(out=xl, in_=xl_d[:, sl].rearrange("(c p) n -> p c n", p=P128))
                xtiles.append((xh, xl))

            def chunk_pieces(t8):
                """Phase-1 for chunk t8 as small closures (PE-gap fillers)."""
                sl = slice(t8 * 512, (t8 + 1) * 512)
                xh, xl = xtiles[t8]
                st = {}

                def kv_main():
                    st["kvps"] = ps_p.tile([2 * D, 512], f32, tag="pp", name="kvps")
                    for c in range(4):
                        nc.tensor.matmul(st["kvps"], wkvh[:, c, :], xh[:, c, :],
                                         start=(c == 0), stop=False)

                def kv_resid():
                    for c in range(4):
                        nc.tensor.matmul(st["kvps"], wkvb[:, c, :], xl[:, c, :],
                                         start=False, stop=(c == 3))

                def kv_drain():
                    nc.scalar.activation(KH[:D, sl], st["kvps"][:D, :],
                                         ACTF.Identity, bias=bks, scale=1.0)
                    vtmp = work.tile([D, 512], fp16, tag="vtmp")
                    nc.scalar.activation(vtmp, st["kvps"][D:, :], ACTF.Identity,
                                         bias=bvs, scale=1.0)
                    for i in range(4):
                        t = t8 * 4 + i
                        nc.sync.dma_start(out=Vv[:, t, :D],
                                          in_=vtmp[:, i * P128:(i + 1) * P128],
                                          transpose=True)

                def q_main():
                    st["qps"] = ps_p.tile([D, 256], f32, tag="pp", name="qps")
                    xh5 = xh.rearrange("p c (s u n) -> p c s u n", u=2, n=P128)
                    for c in range(4):
                        nc.tensor.matmul(st["qps"], wqh[:, c, :],
                                         xh5[:, c, :, 0, :],
                                         start=(c == 0), stop=False)

                def q_resid():
                    xl5 = xl.rearrange("p c (s u n) -> p c s u n", u=2, n=P128)
                    for c in range(4):
                        nc.tensor.matmul(st["qps"], wqb[:, c, :],
                                         xl5[:, c, :, 0, :],
                                         start=False, stop=(c == 3))
                    nc.scalar.activation(QH[:D, t8 * 256:(t8 + 1) * 256],
                                         st["qps"], ACTF.Identity, bias=bqs,
                                         scale=1.0)

                return [kv_main, kv_resid, kv_drain, q_main, q_resid]

            def emit_chunk(t8):
                for p in chunk_pieces(t8):
                    p()

            # ---- phase 2 helpers ----
            def maxpass_pieces(i, st):
                """Row maxes for slots 2i, 2i+1 -> st['mrun2'] (bf16)."""
                pieces = []

                def alloc():
                    st["mrun2"] = small.tile([P128, 2], bf16, tag="mrun2", name="mrun2")

                pieces.append(alloc)
                for s in range(2):
                    j = 2 * i + s
                    ext = 256 * (j + 1)
                    nch = (ext + 511) // 512

                    def mgp_alloc(s=s):
                        st[f"mgp{s}"] = small.tile([P128, 8], f32, tag="mgp", name="mgp")

                    pieces.append(mgp_alloc)
                    for ci in range(nch):
                        def one(s=s, j=j, ci=ci, ext=ext, nch=nch):
                            mgp = st[f"mgp{s}"]
                            soff = ci * 512
                            sw = min(512, ext - soff)
                            last = ci == nch - 1
                            ps = ps_s.tile([P128, 512], f32, tag="ms")
                            nc.tensor.matmul(
                                ps[:, :sw],
                                QH[:D, j * P128:(j + 1) * P128],
                                KH[:D, soff:soff + sw],
                                start=True, stop=not last,
                                skip_group_check=True)
                            if last:
                                # causal mask for the final 256 key cols, as a
                                # matmul: trilq[r,q]=(q<=r<127 or r==127),
                                # m4 rows carry -1e30 (tri at r=pos-1, full
                                # at r=127); accumulates into the same bank.
                                nc.tensor.matmul(
                                    ps[:, :sw], trilq, m4[:, 512 - sw:],
                                    start=False, stop=True,
                                    skip_group_check=True)
                            nc.vector.reduce_max(mgp[:, ci:ci + 1],
                                                 ps[:, :sw], axis=AX)
                            if ci == nch - 1:
                                if nch > 1:
                                    nc.vector.reduce_max(
                                        st["mrun2"][:, s:s + 1],
                                        mgp[:, :nch], axis=AX)
                                else:
                                    nc.vector.tensor_copy(
                                        st["mrun2"][:, s:s + 1], mgp[:, :1])

                        pieces.append(one)
                return pieces

            def emit_maxpass(i):
                st = {}
                for p in maxpass_pieces(i, st):
                    p()
                return st["mrun2"]

            def emit_mfin(i, mrun2):
                """-m into QH row 64 for pair i (PE transpose + DVE copy).
                The matmul writes at partition 64 directly (tile_position) so
                the DVE copy into QH row 64 stays partition-aligned."""
                pm = ps_s.tile([D + 1, 256], f32, tag="ms")
                for s in range(2):
                    nc.tensor.matmul(pm[D:D + 1, s * P128:(s + 1) * P128],
                                     mrun2[:, s:s + 1], idn,
                                     start=True, stop=True,
                                     skip_group_check=True)
                nc.vector.tensor_copy(QH[D:D + 1, i * 256:(i + 1) * 256],
                                      pm[D:D + 1, :])

            def emit_scT(i, w, last):
                ps2 = ps_r.tile([P128, 1024], f32, tag="ring")
                for t in range(4):
                    kt = w * 4 + t
                    nc.tensor.matmul(
                        ps2[:, t * 256:(t + 1) * 256],
                        KH[:, kt * P128:(kt + 1) * P128],
                        QH[:, i * 256:(i + 1) * 256],
                        start=True, stop=not last, skip_group_check=True)
                    if last:
                        nc.tensor.matmul(
                            ps2[:, t * 256:(t + 1) * 256],
                            tril, m2[:, t, :],
                            start=False, stop=True, skip_group_check=True)
                return ps2

            def emit_exp(ps2):
                AT = work.tile([P128, 1024], fp16, tag="AT")
                nc.scalar.activation(AT, ps2, ACTF.Exp, bias=0.0, scale=8.0)
                return AT

            def emit_av(i, w, AT, po2):
                # po2's two slots share one PSUM bank (zero region): exactly
                # one start (zeroes the bank) and one stop for the whole group.
                for t in range(4):
                    gt = w * 4 + t
                    for s in range(2):
                        nc.tensor.matmul(
                            po2[:, s, :],
                            AT[:, t * 256 + s * P128:t * 256 + (s + 1) * P128],
                            Vv[:, gt, :D + 1],
                            start=(w == 0 and t == 0 and s == 0),
                            stop=(w == i and t == 3 and s == 1),
                            skip_group_check=True)

            def emit_yout(i, po2):
                rl = work.tile([P128, 2], f32, tag="rl")
                nc.vector.reciprocal(rl, po2[:, :, D])
                yt = work.tile([P128, 2, D], f32, tag="yt")
                for s in range(2):
                    nc.vector.tensor_scalar_mul(yt[:, s, :], po2[:, s, :D],
                                                rl[:, s:s + 1])
                nc.sync.dma_start(
                    out=y_d[i * 256:(i + 1) * 256, :].rearrange(
                        "(s p) d -> p s d", p=P128),
                    in_=yt)

            # ---- emission: software-pipelined. Next pair's proj+maxpass
            # pieces fill the PE gaps between scT(w) and AV(w-1) (which waits
            # on exp(w-1)); DVE max reduces overlap the DMA-bound head. ----
            def emit_windows(i, fillers):
                po2 = ps_o.tile([P128, 2, D + 1], f32, tag="po2")
                ATs = {}
                for w in range(i + 1):
                    ps2 = emit_scT(i, w, last=(w == i))
                    ATs[w] = emit_exp(ps2)
                    take = (len(fillers) + i - w) // (i + 1 - w)
                    for _ in range(take):
                        fillers.pop(0)()
                    if w >= 1:
                        emit_av(i, w - 1, ATs.pop(w - 1), po2)
                for p in fillers:
                    p()
                emit_av(i, i, ATs.pop(i), po2)
                emit_yout(i, po2)

            for t8 in range(8):
                emit_xload(t8)
            emit_chunk(0)
            mr = emit_maxpass(0)
            emit_mfin(0, mr)
            for i in range(NPAIR):
                if i + 1 < NPAIR:
                    nst = {}
                    fillers = chunk_pieces(i + 1) + maxpass_pieces(i + 1, nst)
                    fillers.append(
                        lambda i=i, nst=nst: emit_mfin(i + 1, nst["mrun2"]))
                else:
                    fillers = []
                emit_windows(i, fillers)

    nc.compile()
    return nc


def _get_nc():
    if "nc" not in _CACHED:
        _CACHED["nc"] = _build()
    return _CACHED["nc"]


def _host_consts():
    import ml_dtypes

    # max-pass mask operand m4 [128, 512] (per parity): the mask for the
    # final 256 key cols of each slot's extent is applied on the PE as
    # trilq^T @ m4.  trilq[r, q] = 1 if (q <= r < 127) or r == 127.
    # tri channel: key-pos p masked for q < p -> m4[p-1, 256+p] = NEG.
    # full channel (row 127): cols 384:512 NEG for P=0 (future tile).
    m4s = []
    for P in range(2):
        m4 = np.zeros((P128, 512), np.float32)
        for p in range(1, P128):
            m4[p - 1, 256 + p] = NEG
        if P == 0:
            m4[P128 - 1, 384:512] = NEG
        m4s.append(_round12(m4))
    trilq = np.zeros((P128, P128), np.float32)
    for r in range(P128):
        for q in range(P128):
            if (q <= r < P128 - 1) or r == P128 - 1:
                trilq[r, q] = 1.0

    # M2 [128, 4, 256]: boundary-window mask operand. Window key-tile
    # position t holds global tile rk = t (P=0) or t^1 (P=1); col half h
    # is global tile rg = 2h + P (relative to 4i).  rk == rg -> tri channel
    # (row r = col_pos, strict); rk > rg -> full channel (row 127).
    m2s = []
    for P in range(2):
        m2 = np.zeros((P128, 4, 256), np.float32)
        for t in range(4):
            rk = t ^ 1 if P == 1 else t
            for h in range(2):
                rg = 2 * h + P
                cs = slice(h * P128, (h + 1) * P128)
                if rk == rg:
                    for r in range(P128 - 1):
                        m2[r, t, h * P128 + r] = NEG
                    # col_pos 127: no key s > 127 in-tile; row 127 reserved
                elif rk > rg:
                    m2[P128 - 1, t, cs] = NEG
        m2s.append(_round12(m2))

    trils = np.zeros((P128, P128), np.float32)
    for r in range(P128):
        for s in range(P128):
            if r < s or r == P128 - 1:
                trils[r, s] = 1.0
    idn = (-np.eye(P128, dtype=np.float32)).astype(ml_dtypes.bfloat16)
    return m4s, trilq, m2s, trils, idn


def _prep_in_maps(x, Wq, bq, Wk, bk, Wv, bv):
    import ml_dtypes

    x = np.asarray(x, dtype=np.float32)
    Wq = np.asarray(Wq, dtype=np.float32)
    Wk = np.asarray(Wk, dtype=np.float32)
    Wv = np.asarray(Wv, dtype=np.float32)
    bq_ = np.asarray(bq, dtype=np.float32).reshape(D, 1)
    bk_ = np.asarray(bk, dtype=np.float32).reshape(D, 1)
    bv_ = np.asarray(bv, dtype=np.float32).reshape(D, 1)

    wkv = np.concatenate([Wk, Wv], axis=1)          # [C, 2D]
    wkvh = _round12(wkv)
    wkvb = wkvh.astype(ml_dtypes.bfloat16)
    wqh = _round12(Wq)
    wqb = wqh.astype(ml_dtypes.bfloat16)

    m4s, trilq, m2s, trils, idn = _host_consts()
    onek = np.ones((1, T), np.float32)
    onev = np.ones((P128, T // P128), np.float16)

    in_maps = []
    for c in range(8):
        b, P = c // 2, c % 2
        xT = np.ascontiguousarray(x[b].T)           # [C, T]
        if P == 1:
            xs = xT.reshape(C, T // 256, 2, P128)[:, :, ::-1, :]
            xT = np.ascontiguousarray(xs.reshape(C, T))
        xh = _round12(xT)
        xl = (xT - xh).astype(ml_dtypes.bfloat16)
        in_maps.append({
            "xh": xh, "xl": np.ascontiguousarray(xl),
            "wkvh": wkvh, "wkvb": np.ascontiguousarray(wkvb),
            "wqh": wqh, "wqb": np.ascontiguousarray(wqb),
            "bq": bq_, "bk": bk_, "bv": bv_,
            "m4": m4s[P], "trilq": trilq, "m2": m2s[P],
            "tril": trils, "idn": idn,
            "onek": onek, "onev": onev,
        })
    return in_maps


def _unshard(res):
    out = np.empty((B, T, D), np.float32)
    for c in range(8):
        b, P = c // 2, c % 2
        yl = res.results[c]["y"]
        for j in range(NSLOT):
            g = 2 * j + P
            out[b, g * P128:(g + 1) * P128] = yl[j * P128:(j + 1) * P128]
    return out


def kernel(x, Wq, bq, Wk, bk, Wv, bv):
    from concourse.bass_utils import run_bass_kernel_spmd

    in_maps = _prep_in_maps(x, Wq, bq, Wk, bk, Wv, bv)
    res = run_bass_kernel_spmd(_get_nc(), in_maps, core_ids=list(range(8)))
    _CACHED["last_results"] = res
    return _unshard(res)


if __name__ == "__main__":
    rng = np.random.default_rng(0)
    x = rng.standard_normal((B, T, C), dtype=np.float32)
    s = 1.0 / np.sqrt(C)
    Wq = rng.standard_normal((C, D), dtype=np.float32) * s
    Wk = rng.standard_normal((C, D), dtype=np.float32) * s
    Wv = rng.standard_normal((C, D), dtype=np.float32) * s
    z = np.zeros(D, np.float32)
    print(kernel(x, Wq, z, Wk, z, Wv, z).shape)
